# revision 33
# baseline (speedup 1.0000x reference)
"""Trainium2 Bass kernel for nn_Attention_5927054869144.

Channel-attention over [B=8, C=64, H=256, W=256] inputs. Data-parallel over
batch: one batch element per NeuronCore (8 cores), no collectives.

Per-core pipeline (x_b viewed as [64, 65536], spatial blocks of 8192):
  1. qkvT projection with x-chunk stationary on the PE -> q/k/v directly in
     spatial-partition layout (fp16 operands, fp32 PSUM).
  2. Per-head-pair dots matmuls from gathered (alpha, i) column APs,
     accumulated in PSUM over all spatial tiles (softmax scale folded into
     Wq/bq host-side).
  3. Unnormalized softmax: exp(x - max) on ScalarE with accumulated row sums;
     1/rowsum folded into per-head copies of Wo^T.
  4. P_h^T = expdots_h @ (Wo^T * recip) written into a block-diagonal
     [128,128] lhsT per head pair.
  5. v transposed to dim-partition layout via TensorE gather-transposes
     (fp16 PSUM), then final output = blockdiag(P)^T @ v_dp in single K=128
     matmuls, evacuated fp16 and written back via engine-rotated SWDGE DMAs.

I/O is fp16 on the wire: x is cast + quarter-major reordered host-side (so
input DMA descriptors are 32KB/partition), output is written fp16 and
upcast host-side.
"""

import os
import sys

import numpy as np

for _p in ("/opt/trn_rl_repo", "/root/.axon_site/_ro/trn_rl_repo"):
    if os.path.isdir(_p) and _p not in sys.path:
        sys.path.insert(0, _p)

from concourse import bacc, mybir, tile  # noqa: E402
from concourse.bass_utils import run_bass_kernel_spmd  # noqa: E402

F32 = mybir.dt.float32
F16 = mybir.dt.float16

HEADS = 8
C = 64
HW = 65536          # 256*256 spatial positions per batch element
BL = HW // HEADS    # 8192, per-head block length
NQ = 4              # spatial quarters (within-block n ranges)
QL = BL // NQ       # 2048 within-block positions per quarter
TPQ = QL // 128     # 16 tile groups per quarter
N_GROUPS = BL // 128  # 64 total tile groups

LAST_RESULTS = None


def _build_kernel(hw=HW):
    bl = hw // HEADS
    ql = bl // NQ
    tpq = ql // 128
    n_groups = bl // 128

    nc = bacc.Bacc("TRN2", target_bir_lowering=False, debug=False)
    # x columns are quarter-major host-side: (q, i, ql)
    x_d = nc.dram_tensor("x", [65, hw], F16, kind="ExternalInput")
    wqkv_d = nc.dram_tensor("wqkv", [65, 192], F16, kind="ExternalInput")
    wot_d = nc.dram_tensor("wot", [128, 64], F32, kind="ExternalInput")
    ident_d = nc.dram_tensor("ident", [128, 128], F16, kind="ExternalInput")
    out_d = nc.dram_tensor("out", [64, hw], F16, kind="ExternalOutput")

    x_ap = x_d.ap()
    out_ap = out_d.ap()
    # [pr, s, o, n] view of the output: head h = 2*pr + s
    out_v = out_ap.rearrange("o (p s n) -> p s o n", p=4, s=2)

    with tile.TileContext(nc) as tc:
        with (
            tc.tile_pool(name="consts", bufs=1) as cpool,
            tc.tile_pool(name="pers", bufs=1) as pers,
            tc.tile_pool(name="smx", bufs=1) as smx,
            tc.tile_pool(name="dotsp", bufs=1, space="PSUM") as dotspool,
        ):
            wqkv_sb = cpool.tile([65, 192], F16)
            wot_sb = cpool.tile([128, 64], F32)
            ident_sb = cpool.tile([128, 128], F16)
            nc.sync.dma_start(out=wqkv_sb[:, :], in_=wqkv_d.ap()[:, :])
            nc.sync.dma_start(out=wot_sb[:, :], in_=wot_d.ap()[:, :])
            nc.sync.dma_start(out=ident_sb[:, :], in_=ident_d.ap()[:, :])

            # v in dim-partition layout: [pair, d(0:64 even head / 64:128 odd), n]
            vdp = pers.tile([128, 4 * bl], F16)
            # all four head-pair dot accumulators share one PSUM bank.
            # start=True clears has_written BANK-wide, so only the very first
            # matmul may set it; later pairs' first writes see hw=0 and
            # overwrite, which is the correct group-begin behavior.
            dots_all = dotspool.tile([128, 512], F32, name="dots")
            dots_ps = [dots_all[:, 128 * p: 128 * p + 128] for p in range(4)]

            # block-diagonal P^T per head pair; off-diag blocks stay zero
            mhbd = [smx.tile([128, 128], F16, name=f"mhbd{p}") for p in range(4)]
            for p in range(4):
                nc.vector.memset(mhbd[p][:, :], 0.0)

            # PE warmup during the input-load head: back-to-back matmuls flip
            # the HAM clock gate to 8/8 (2.4 GHz) and keep it there until the
            # first projection matmuls are ready — any >3.4us PE-idle gap
            # re-throttles, and ~80%-busy phase A never re-warms it. Results
            # land in dots PSUM and are overwritten by the g==0 matmuls
            # (hw bits cleared by its start=True).
            for _ in range(100):
                nc.tensor.matmul(
                    dots_ps[0][:, :],
                    lhsT=ident_sb[:, :],
                    rhs=ident_sb[:, :],
                    start=True,
                    stop=True,
                )

            # ---------------- Phase A ----------------
            vdp_v = vdp.rearrange("p (r n) -> p r n", r=4)
            with (
                tc.tile_pool(name="xq", bufs=2) as xpool,
                tc.tile_pool(name="slots", bufs=5) as slotpool,
                tc.tile_pool(name="projp", bufs=4, space="PSUM") as projpool,
                tc.tile_pool(name="vtrp", bufs=3, space="PSUM") as vtrpool,
            ):
                slots = {}

                def ecopy(k, dst, src):
                    # PSUM evacuation: only DVE/ACT can read PSUM
                    if k % 2 == 0:
                        nc.vector.tensor_copy(dst, src)
                    else:
                        nc.scalar.copy(dst, src)

                def consume(g):
                    # dots + v-transpose for a group whose slot is fully evac'd
                    slot = slots.pop(g)
                    vt = vtrpool.tile([128, 512], F16, name="vt")
                    for pr in range(4):
                        qs = slot[:, 128 * pr: 128 * pr + 128]
                        ks = slot[:, 512 + 128 * pr: 512 + 128 * pr + 128]
                        vs = slot[:, 1024 + 128 * pr: 1024 + 128 * pr + 128]
                        nc.tensor.matmul(
                            dots_ps[pr][:, :],
                            lhsT=qs,
                            rhs=ks,
                            start=(g == 0 and pr == 0),
                            stop=(g == n_groups - 1),
                        )
                        nc.tensor.transpose(
                            vt[:, pr * 128:(pr + 1) * 128], vs, ident_sb[:, :]
                        )
                    voff = g * 128
                    ecopy(g + 1, vdp_v[:, :, voff:voff + 128], vt[:, :])

                sq = 8 * ql // 4  # sub-load columns (4 per quarter)
                for q in range(NQ):
                    xq = xpool.tile([65, 8 * ql], F16, name="xq")
                    # t-major DRAM layout: 4 sub-loads per quarter so the
                    # first tile groups can start before the quarter lands
                    for tq in range(4):
                        nc.sync.dma_start(
                            out=xq[:, tq * sq:(tq + 1) * sq],
                            in_=x_ap[:, q * 8 * ql + tq * sq:
                                     q * 8 * ql + (tq + 1) * sq],
                        )
                    for t0 in range(tpq):
                        g = q * tpq + t0
                        tq, t0r = t0 // 4, t0 % 4
                        # slot cols: r*512 + head*64 + i*8 + alpha (alpha contiguous)
                        slot = slotpool.tile([128, 1536], F16, name="slot")
                        slot_sc = slot.rearrange(
                            "p (r h i a) -> p i r h a", r=3, h=8, i=8, a=8
                        )
                        slots[g] = slot
                        for ip in range(4):  # chunk pairs (2i, 2i+1)
                            pp = projpool.tile([128, 384], F32, name="pp")
                            for c in range(2):
                                i = 2 * ip + c
                                x0 = tq * sq + i * 512 + t0r * 128
                                nc.tensor.matmul(
                                    pp[:, c * 192:(c + 1) * 192],
                                    lhsT=xq[:, x0:x0 + 128],
                                    rhs=wqkv_sb[:, :],
                                    start=True,
                                    stop=True,
                                )
                            dst = slot_sc[:, 2 * ip: 2 * ip + 2, :, :, :]
                            ecopy(ip, dst, pp[:, :])
                        if g >= 3:
                            consume(g - 3)
                for g in (n_groups - 3, n_groups - 2, n_groups - 1):
                    consume(g)

            # ---------------- Softmax + output ----------------
            with (
                tc.tile_pool(name="mhp", bufs=1, space="PSUM") as mhpool,
                tc.tile_pool(name="finp", bufs=5, space="PSUM") as finpool,
                tc.tile_pool(name="outs", bufs=2) as outpool,
            ):
                # keep the PE hot across the softmax gap so the finals run
                # at 2.4 GHz (issued right after the last dots matmul)
                warm = mhpool.tile([128, 128], F32, name="warm")
                for _ in range(40):
                    nc.tensor.matmul(
                        warm[:, :],
                        lhsT=ident_sb[:, :],
                        rhs=ident_sb[:, :],
                        start=True,
                        stop=True,
                    )

                negmax = smx.tile([128, 4], F32)
                rowsum = smx.tile([128, 4], F32)
                exps = smx.tile([128, 4 * 64], F16)
                wots = smx.tile([128, 4 * 64], F16)
                mh_ps = mhpool.tile([128, 4 * 64], F32)

                def hpb(h):
                    return h // 2, (h % 2) * 64

                # batched per-op issue so the cross-engine chain pipelines
                for h in range(HEADS):
                    pr, b = hpb(h)
                    nc.vector.reduce_max(
                        negmax[b:b + 64, pr:pr + 1],
                        dots_ps[pr][b:b + 64, b:b + 64],
                        axis=mybir.AxisListType.X, negate=True,
                    )
                for h in range(HEADS):
                    pr, b = hpb(h)
                    nc.scalar.activation(
                        exps[b:b + 64, pr * 64:(pr + 1) * 64],
                        dots_ps[pr][b:b + 64, b:b + 64],
                        mybir.ActivationFunctionType.Exp,
                        bias=negmax[b:b + 64, pr:pr + 1],
                        scale=1.0,
                        accum_out=rowsum[b:b + 64, pr:pr + 1],
                    )
                recip = smx.tile([128, 4], F32)
                for h in range(HEADS):
                    pr, b = hpb(h)
                    nc.vector.reciprocal(
                        recip[b:b + 64, pr:pr + 1], rowsum[b:b + 64, pr:pr + 1]
                    )
                for h in range(HEADS):
                    pr, b = hpb(h)
                    # 1/rowsum folded into the per-head copy of Wo^T
                    nc.vector.tensor_scalar_mul(
                        wots[b:b + 64, pr * 64:(pr + 1) * 64],
                        wot_sb[b:b + 64, :],
                        recip[b:b + 64, pr:pr + 1],
                    )
                for pr in range(4):
                    for s in range(2):
                        b = s * 64
                        nc.tensor.matmul(
                            mh_ps[b:b + 64, pr * 64:(pr + 1) * 64],
                            lhsT=exps[b:b + 64, pr * 64:(pr + 1) * 64],
                            rhs=wots[b:b + 64, pr * 64:(pr + 1) * 64],
                            start=True,
                            stop=True,
                        )
                        src = mh_ps[b:b + 64, pr * 64:(pr + 1) * 64]
                        dst = mhbd[pr][b:b + 64, b:b + 64]
                        if s == 0:
                            nc.vector.tensor_copy(dst, src)
                        else:
                            nc.scalar.copy(dst, src)

                    outsb = outpool.tile([128, bl], F16, name="outsb")
                    for s5 in range(bl // 512):
                        fp_ = finpool.tile([128, 512], F32, name="fp_")
                        n0 = pr * bl + s5 * 512
                        nc.tensor.matmul(
                            fp_[:, :],
                            lhsT=mhbd[pr][:, :],
                            rhs=vdp[:, n0:n0 + 512],
                            start=True,
                            stop=True,
                        )
                        dst = outsb[:, s5 * 512:(s5 + 1) * 512]
                        if s5 % 2 == 0:
                            nc.vector.tensor_copy(dst, fp_[:, :])
                        else:
                            nc.scalar.copy(dst, fp_[:, :])
                    # 4 SWDGE stores of 32 descriptors each: the packet
                    # rotation spreads consecutive calls across SDMA engines
                    for j in range(4):
                        s, o0 = j // 2, (j % 2) * 32
                        nc.gpsimd.dma_start(
                            out=out_v[pr, s, o0:o0 + 32, :],
                            in_=outsb[64 * s + o0: 64 * s + o0 + 32, :],
                        )

    nc.compile()
    return nc


_NC_CACHE = {}


def _get_nc(hw=HW):
    if hw not in _NC_CACHE:
        _NC_CACHE[hw] = _build_kernel(hw)
    return _NC_CACHE[hw]


def _host_inputs(Wq, bq, Wk, bk, Wv, bv, Wo):
    scale = 64 ** -0.5
    wqkv = np.zeros((65, 192), np.float16)
    wqkv[:64, 0:64] = (Wq.T * scale).astype(np.float16)
    wqkv[64, 0:64] = (bq * scale).astype(np.float16)
    wqkv[:64, 64:128] = Wk.T.astype(np.float16)
    wqkv[64, 64:128] = bk.astype(np.float16)
    wqkv[:64, 128:192] = Wv.T.astype(np.float16)
    wqkv[64, 128:192] = bv.astype(np.float16)
    # kernel uses c' = i*8 + alpha ordering; original c = alpha*8 + i
    pi = np.array([(c % 8) * 8 + c // 8 for c in range(64)])
    wotp = Wo.T[pi]
    wot = np.concatenate([wotp, wotp], axis=0).astype(np.float32)
    ident = np.eye(128, dtype=np.float16)
    return wqkv, wot, ident


def kernel(x, Wq, bq, Wk, bk, Wv, bv, Wo):
    global LAST_RESULTS
    B = x.shape[0]
    hw = x.shape[2] * x.shape[3]
    nc = _get_nc(hw)
    wqkv, wot, ident = _host_inputs(Wq, bq, Wk, bk, Wv, bv, Wo)

    ql = hw // HEADS // NQ
    in_maps = []
    for bidx in range(B):
        x65 = np.empty((65, hw), np.float16)
        x65[:64] = x[bidx].reshape(64, hw)
        x65[64] = 1.0
        # (q, tq, i, 512) column order: each quarter is 4 contiguous
        # sub-loads of 8KB-per-partition descriptors
        x65 = np.ascontiguousarray(
            x65.reshape(65, HEADS, NQ, 4, ql // 4).transpose(0, 2, 3, 1, 4)
        ).reshape(65, hw)
        in_maps.append({"x": x65, "wqkv": wqkv, "wot": wot, "ident": ident})

    trace = bool(os.environ.get("KERNEL_TRACE"))
    res = run_bass_kernel_spmd(
        nc, in_maps, core_ids=list(range(B)), trace=trace
    )
    LAST_RESULTS = res
    out = np.stack(
        [res.results[bidx]["out"].reshape(64, HEADS, hw // HEADS)
         for bidx in range(B)]
    ).astype(np.float32)
    return out


# revision 37
# speedup vs baseline: 1.0265x; 1.0265x over previous
"""Trainium2 Bass kernel for nn_Attention_5927054869144.

Channel-attention over [B=8, C=64, H=256, W=256] inputs. Data-parallel over
batch: one batch element per NeuronCore (8 cores), no collectives.

Per-core pipeline (x_b viewed as [64, 65536], spatial blocks of 8192):
  1. qkvT projection with x-chunk stationary on the PE -> q/k/v directly in
     spatial-partition layout (fp16 operands, fp32 PSUM).
  2. Per-head-pair dots matmuls from gathered (alpha, i) column APs,
     accumulated in PSUM over all spatial tiles (softmax scale folded into
     Wq/bq host-side).
  3. Unnormalized softmax: exp(x - max) on ScalarE with accumulated row sums;
     1/rowsum folded into per-head copies of Wo^T.
  4. P_h^T = expdots_h @ (Wo^T * recip) written into a block-diagonal
     [128,128] lhsT per head pair.
  5. v transposed to dim-partition layout via TensorE gather-transposes
     (fp16 PSUM), then final output = blockdiag(P)^T @ v_dp in single K=128
     matmuls, evacuated fp16 and written back via engine-rotated SWDGE DMAs.

I/O is fp16 on the wire: x is cast + quarter-major reordered host-side (so
input DMA descriptors are 32KB/partition), output is written fp16 and
upcast host-side.
"""

import os
import sys

import numpy as np

for _p in ("/opt/trn_rl_repo", "/root/.axon_site/_ro/trn_rl_repo"):
    if os.path.isdir(_p) and _p not in sys.path:
        sys.path.insert(0, _p)

from concourse import bacc, mybir, tile  # noqa: E402
from concourse.bass_utils import run_bass_kernel_spmd  # noqa: E402

F32 = mybir.dt.float32
F16 = mybir.dt.float16

HEADS = 8
C = 64
HW = 65536          # 256*256 spatial positions per batch element
BL = HW // HEADS    # 8192, per-head block length
NQ = 4              # spatial quarters (within-block n ranges)
QL = BL // NQ       # 2048 within-block positions per quarter
TPQ = QL // 128     # 16 tile groups per quarter
N_GROUPS = BL // 128  # 64 total tile groups

LAST_RESULTS = None


def _build_kernel(hw=HW):
    bl = hw // HEADS
    ql = bl // NQ
    tpq = ql // 128
    n_groups = bl // 128

    nc = bacc.Bacc("TRN2", target_bir_lowering=False, debug=False)
    # x columns are quarter-major host-side: (q, i, ql)
    x_d = nc.dram_tensor("x", [65, hw], F16, kind="ExternalInput")
    wqkv_d = nc.dram_tensor("wqkv", [65, 192], F16, kind="ExternalInput")
    wot_d = nc.dram_tensor("wot", [128, 64], F32, kind="ExternalInput")
    ident_d = nc.dram_tensor("ident", [128, 128], F16, kind="ExternalInput")
    out_d = nc.dram_tensor("out", [64, hw], F16, kind="ExternalOutput")

    x_ap = x_d.ap()
    out_ap = out_d.ap()
    # [pr, s, o, n] view of the output: head h = 2*pr + s
    out_v = out_ap.rearrange("o (p s n) -> p s o n", p=4, s=2)

    with tile.TileContext(nc) as tc:
        with (
            tc.tile_pool(name="consts", bufs=1) as cpool,
            tc.tile_pool(name="pers", bufs=1) as pers,
            tc.tile_pool(name="smx", bufs=1) as smx,
            tc.tile_pool(name="dotsp", bufs=1, space="PSUM") as dotspool,
        ):
            wqkv_sb = cpool.tile([65, 192], F16)
            wot_sb = cpool.tile([128, 64], F32)
            ident_sb = cpool.tile([128, 128], F16)
            warmsrc = cpool.tile([128, 512], F16)
            nc.vector.memset(warmsrc[:, :], 0.0)
            nc.sync.dma_start(out=wqkv_sb[:, :], in_=wqkv_d.ap()[:, :])
            nc.sync.dma_start(out=wot_sb[:, :], in_=wot_d.ap()[:, :])
            nc.sync.dma_start(out=ident_sb[:, :], in_=ident_d.ap()[:, :])

            # v in dim-partition layout: [pair, d(0:64 even head / 64:128 odd), n]
            vdp = pers.tile([128, 4 * bl], F16)
            # all four head-pair dot accumulators share one PSUM bank.
            # start=True clears has_written BANK-wide, so only the very first
            # matmul may set it; later pairs' first writes see hw=0 and
            # overwrite, which is the correct group-begin behavior.
            dots_all = dotspool.tile([128, 512], F32, name="dots")
            dots_ps = [dots_all[:, 128 * p: 128 * p + 128] for p in range(4)]

            # block-diagonal P^T per head pair; off-diag blocks stay zero
            mhbd = [smx.tile([128, 128], F16, name=f"mhbd{p}") for p in range(4)]
            for p in range(4):
                nc.vector.memset(mhbd[p][:, :], 0.0)

            # PE warmup during the input-load head: back-to-back matmuls flip
            # the HAM clock gate to 8/8 (2.4 GHz) and keep it there until the
            # first projection matmuls are ready — any >3.4us PE-idle gap
            # re-throttles, and ~80%-busy phase A never re-warms it. Results
            # land in dots PSUM and are overwritten by the g==0 matmuls
            # (hw bits cleared by its start=True).
            for _ in range(34):
                nc.tensor.matmul(
                    dots_all[:, :],
                    lhsT=warmsrc[:, 0:128],
                    rhs=warmsrc[:, :],
                    start=True,
                    stop=True,
                )

            # ---------------- Phase A ----------------
            vdp_v = vdp.rearrange("p (r n) -> p r n", r=4)
            with (
                tc.tile_pool(name="xq", bufs=2) as xpool,
                tc.tile_pool(name="slots", bufs=5) as slotpool,
                tc.tile_pool(name="projp", bufs=4, space="PSUM") as projpool,
                tc.tile_pool(name="vtrp", bufs=3, space="PSUM") as vtrpool,
            ):
                slots = {}

                def ecopy(k, dst, src):
                    # PSUM evacuation: only DVE/ACT can read PSUM
                    if k % 2 == 0:
                        nc.vector.tensor_copy(dst, src)
                    else:
                        nc.scalar.copy(dst, src)

                def consume(g):
                    # dots + v-transpose for a group whose slot is fully evac'd
                    slot = slots.pop(g)
                    vt = vtrpool.tile([128, 512], F16, name="vt")
                    for pr in range(4):
                        qs = slot[:, 128 * pr: 128 * pr + 128]
                        ks = slot[:, 512 + 128 * pr: 512 + 128 * pr + 128]
                        vs = slot[:, 1024 + 128 * pr: 1024 + 128 * pr + 128]
                        nc.tensor.matmul(
                            dots_ps[pr][:, :],
                            lhsT=qs,
                            rhs=ks,
                            start=(g == 0 and pr == 0),
                            stop=(g == n_groups - 1),
                        )
                        nc.tensor.transpose(
                            vt[:, pr * 128:(pr + 1) * 128], vs, ident_sb[:, :]
                        )
                    voff = g * 128
                    ecopy(g + 1, vdp_v[:, :, voff:voff + 128], vt[:, :])

                sq = 8 * ql // 4  # sub-load columns (4 per quarter)
                for q in range(NQ):
                    xq = xpool.tile([65, 8 * ql], F16, name="xq")
                    # t-major DRAM layout: 4 sub-loads per quarter so the
                    # first tile groups can start before the quarter lands
                    for tq in range(4):
                        nc.sync.dma_start(
                            out=xq[:, tq * sq:(tq + 1) * sq],
                            in_=x_ap[:, q * 8 * ql + tq * sq:
                                     q * 8 * ql + (tq + 1) * sq],
                        )
                    for t0 in range(tpq):
                        g = q * tpq + t0
                        tq, t0r = t0 // 4, t0 % 4
                        # slot cols: r*512 + head*64 + i*8 + alpha (alpha contiguous)
                        slot = slotpool.tile([128, 1536], F16, name="slot")
                        slot_sc = slot.rearrange(
                            "p (r h i a) -> p i r h a", r=3, h=8, i=8, a=8
                        )
                        slots[g] = slot
                        for ip in range(4):  # chunk pairs (2i, 2i+1)
                            pp = projpool.tile([128, 384], F32, name="pp")
                            for c in range(2):
                                i = 2 * ip + c
                                x0 = tq * sq + i * 512 + t0r * 128
                                nc.tensor.matmul(
                                    pp[:, c * 192:(c + 1) * 192],
                                    lhsT=xq[:, x0:x0 + 128],
                                    rhs=wqkv_sb[:, :],
                                    start=True,
                                    stop=True,
                                )
                            dst = slot_sc[:, 2 * ip: 2 * ip + 2, :, :, :]
                            ecopy(ip, dst, pp[:, :])
                        if g >= 3:
                            consume(g - 3)
                for g in (n_groups - 3, n_groups - 2, n_groups - 1):
                    consume(g)

            # ---------------- Softmax + output ----------------
            with (
                tc.tile_pool(name="mhp", bufs=1, space="PSUM") as mhpool,
                tc.tile_pool(name="finp", bufs=5, space="PSUM") as finpool,
                tc.tile_pool(name="outs", bufs=2) as outpool,
            ):
                # keep the PE hot across the softmax gap so the finals run
                # at 2.4 GHz (issued right after the last dots matmul)
                warm = mhpool.tile([128, 256], F32, name="warm")
                for _ in range(26):
                    nc.tensor.matmul(
                        warm[:, :],
                        lhsT=warmsrc[:, 0:128],
                        rhs=warmsrc[:, 0:256],
                        start=True,
                        stop=True,
                    )

                negmax = smx.tile([128, 4], F32)
                rowsum = smx.tile([128, 4], F32)
                exps = smx.tile([128, 4 * 64], F16)
                wots = smx.tile([128, 4 * 64], F16)
                mh_ps = mhpool.tile([128, 4 * 64], F32)

                def hpb(h):
                    return h // 2, (h % 2) * 64

                # batched per-op issue so the cross-engine chain pipelines
                for h in range(HEADS):
                    pr, b = hpb(h)
                    nc.vector.reduce_max(
                        negmax[b:b + 64, pr:pr + 1],
                        dots_ps[pr][b:b + 64, b:b + 64],
                        axis=mybir.AxisListType.X, negate=True,
                    )
                for h in range(HEADS):
                    pr, b = hpb(h)
                    nc.scalar.activation(
                        exps[b:b + 64, pr * 64:(pr + 1) * 64],
                        dots_ps[pr][b:b + 64, b:b + 64],
                        mybir.ActivationFunctionType.Exp,
                        bias=negmax[b:b + 64, pr:pr + 1],
                        scale=1.0,
                        accum_out=rowsum[b:b + 64, pr:pr + 1],
                    )
                recip = smx.tile([128, 4], F32)
                for h in range(HEADS):
                    pr, b = hpb(h)
                    nc.vector.reciprocal(
                        recip[b:b + 64, pr:pr + 1], rowsum[b:b + 64, pr:pr + 1]
                    )
                for h in range(HEADS):
                    pr, b = hpb(h)
                    # 1/rowsum folded into the per-head copy of Wo^T
                    nc.vector.tensor_scalar_mul(
                        wots[b:b + 64, pr * 64:(pr + 1) * 64],
                        wot_sb[b:b + 64, :],
                        recip[b:b + 64, pr:pr + 1],
                    )
                for pr in range(4):
                    for s in range(2):
                        b = s * 64
                        nc.tensor.matmul(
                            mh_ps[b:b + 64, pr * 64:(pr + 1) * 64],
                            lhsT=exps[b:b + 64, pr * 64:(pr + 1) * 64],
                            rhs=wots[b:b + 64, pr * 64:(pr + 1) * 64],
                            start=True,
                            stop=True,
                        )
                        src = mh_ps[b:b + 64, pr * 64:(pr + 1) * 64]
                        dst = mhbd[pr][b:b + 64, b:b + 64]
                        if s == 0:
                            nc.vector.tensor_copy(dst, src)
                        else:
                            nc.scalar.copy(dst, src)

                    outsb = outpool.tile([128, bl], F16, name="outsb")
                    for s5 in range(bl // 512):
                        fp_ = finpool.tile([128, 512], F32, name="fp_")
                        n0 = pr * bl + s5 * 512
                        nc.tensor.matmul(
                            fp_[:, :],
                            lhsT=mhbd[pr][:, :],
                            rhs=vdp[:, n0:n0 + 512],
                            start=True,
                            stop=True,
                        )
                        dst = outsb[:, s5 * 512:(s5 + 1) * 512]
                        if s5 % 2 == 0:
                            nc.vector.tensor_copy(dst, fp_[:, :])
                        else:
                            nc.scalar.copy(dst, fp_[:, :])
                    # 4 SWDGE stores of 32 descriptors each: the packet
                    # rotation spreads consecutive calls across SDMA engines
                    for j in range(4):
                        s, o0 = j // 2, (j % 2) * 32
                        nc.gpsimd.dma_start(
                            out=out_v[pr, s, o0:o0 + 32, :],
                            in_=outsb[64 * s + o0: 64 * s + o0 + 32, :],
                        )

    nc.compile()
    return nc


_NC_CACHE = {}


def _get_nc(hw=HW):
    if hw not in _NC_CACHE:
        _NC_CACHE[hw] = _build_kernel(hw)
    return _NC_CACHE[hw]


def _host_inputs(Wq, bq, Wk, bk, Wv, bv, Wo):
    scale = 64 ** -0.5
    wqkv = np.zeros((65, 192), np.float16)
    wqkv[:64, 0:64] = (Wq.T * scale).astype(np.float16)
    wqkv[64, 0:64] = (bq * scale).astype(np.float16)
    wqkv[:64, 64:128] = Wk.T.astype(np.float16)
    wqkv[64, 64:128] = bk.astype(np.float16)
    wqkv[:64, 128:192] = Wv.T.astype(np.float16)
    wqkv[64, 128:192] = bv.astype(np.float16)
    # kernel uses c' = i*8 + alpha ordering; original c = alpha*8 + i
    pi = np.array([(c % 8) * 8 + c // 8 for c in range(64)])
    wotp = Wo.T[pi]
    wot = np.concatenate([wotp, wotp], axis=0).astype(np.float32)
    ident = np.eye(128, dtype=np.float16)
    return wqkv, wot, ident


def kernel(x, Wq, bq, Wk, bk, Wv, bv, Wo):
    global LAST_RESULTS
    B = x.shape[0]
    hw = x.shape[2] * x.shape[3]
    nc = _get_nc(hw)
    wqkv, wot, ident = _host_inputs(Wq, bq, Wk, bk, Wv, bv, Wo)

    ql = hw // HEADS // NQ
    in_maps = []
    for bidx in range(B):
        x65 = np.empty((65, hw), np.float16)
        x65[:64] = x[bidx].reshape(64, hw)
        x65[64] = 1.0
        # (q, tq, i, 512) column order: each quarter is 4 contiguous
        # sub-loads of 8KB-per-partition descriptors
        x65 = np.ascontiguousarray(
            x65.reshape(65, HEADS, NQ, 4, ql // 4).transpose(0, 2, 3, 1, 4)
        ).reshape(65, hw)
        in_maps.append({"x": x65, "wqkv": wqkv, "wot": wot, "ident": ident})

    trace = bool(os.environ.get("KERNEL_TRACE"))
    res = run_bass_kernel_spmd(
        nc, in_maps, core_ids=list(range(B)), trace=trace
    )
    LAST_RESULTS = res
    out = np.stack(
        [res.results[bidx]["out"].reshape(64, HEADS, hw // HEADS)
         for bidx in range(B)]
    ).astype(np.float32)
    return out


# revision 38
# speedup vs baseline: 1.0431x; 1.0161x over previous
"""Trainium2 Bass kernel for nn_Attention_5927054869144.

Channel-attention over [B=8, C=64, H=256, W=256] inputs. Data-parallel over
batch: one batch element per NeuronCore (8 cores), no collectives.

Per-core pipeline (x_b viewed as [64, 65536], spatial blocks of 8192):
  1. qkvT projection with x-chunk stationary on the PE -> q/k/v directly in
     spatial-partition layout (fp16 operands, fp32 PSUM).
  2. Per-head-pair dots matmuls from gathered (alpha, i) column APs,
     accumulated in PSUM over all spatial tiles (softmax scale folded into
     Wq/bq host-side).
  3. Unnormalized softmax: exp(x - max) on ScalarE with accumulated row sums;
     1/rowsum folded into per-head copies of Wo^T.
  4. P_h^T = expdots_h @ (Wo^T * recip) written into a block-diagonal
     [128,128] lhsT per head pair.
  5. v transposed to dim-partition layout via TensorE gather-transposes
     (fp16 PSUM), then final output = blockdiag(P)^T @ v_dp in single K=128
     matmuls, evacuated fp16 and written back via engine-rotated SWDGE DMAs.

I/O is fp16 on the wire: x is cast + quarter-major reordered host-side (so
input DMA descriptors are 32KB/partition), output is written fp16 and
upcast host-side.
"""

import os
import sys

import numpy as np

for _p in ("/opt/trn_rl_repo", "/root/.axon_site/_ro/trn_rl_repo"):
    if os.path.isdir(_p) and _p not in sys.path:
        sys.path.insert(0, _p)

from concourse import bacc, mybir, tile  # noqa: E402
from concourse.bass_utils import run_bass_kernel_spmd  # noqa: E402

F32 = mybir.dt.float32
F16 = mybir.dt.float16

HEADS = 8
C = 64
HW = 65536          # 256*256 spatial positions per batch element
BL = HW // HEADS    # 8192, per-head block length
NQ = 4              # spatial quarters (within-block n ranges)
QL = BL // NQ       # 2048 within-block positions per quarter
TPQ = QL // 128     # 16 tile groups per quarter
N_GROUPS = BL // 128  # 64 total tile groups

LAST_RESULTS = None


def _build_kernel(hw=HW):
    bl = hw // HEADS
    ql = bl // NQ
    tpq = ql // 128
    n_groups = bl // 128

    nc = bacc.Bacc("TRN2", target_bir_lowering=False, debug=False)
    # x columns are quarter-major host-side: (q, i, ql)
    x_d = nc.dram_tensor("x", [65, hw], F16, kind="ExternalInput")
    wqkv_d = nc.dram_tensor("wqkv", [65, 192], F16, kind="ExternalInput")
    wot_d = nc.dram_tensor("wot", [128, 64], F32, kind="ExternalInput")
    ident_d = nc.dram_tensor("ident", [128, 128], F16, kind="ExternalInput")
    out_d = nc.dram_tensor("out", [64, hw], F16, kind="ExternalOutput")

    x_ap = x_d.ap()
    out_ap = out_d.ap()
    # [pr, s, o, n] view of the output: head h = 2*pr + s
    out_v = out_ap.rearrange("o (p s n) -> p s o n", p=4, s=2)

    with tile.TileContext(nc) as tc:
        with (
            tc.tile_pool(name="consts", bufs=1) as cpool,
            tc.tile_pool(name="pers", bufs=1) as pers,
            tc.tile_pool(name="smx", bufs=1) as smx,
            tc.tile_pool(name="dotsp", bufs=1, space="PSUM") as dotspool,
        ):
            wqkv_sb = cpool.tile([65, 192], F16)
            wot_sb = cpool.tile([128, 64], F32)
            ident_sb = cpool.tile([128, 128], F16)
            warmsrc = cpool.tile([128, 512], F16)
            nc.vector.memset(warmsrc[:, :], 0.0)
            nc.sync.dma_start(out=wqkv_sb[:, :], in_=wqkv_d.ap()[:, :])
            nc.sync.dma_start(out=wot_sb[:, :], in_=wot_d.ap()[:, :])
            nc.sync.dma_start(out=ident_sb[:, :], in_=ident_d.ap()[:, :])

            # v in dim-partition layout: [pair, d(0:64 even head / 64:128 odd), n]
            vdp = pers.tile([128, 4 * bl], F16)
            # all four head-pair dot accumulators share one PSUM bank.
            # start=True clears has_written BANK-wide, so only the very first
            # matmul may set it; later pairs' first writes see hw=0 and
            # overwrite, which is the correct group-begin behavior.
            dots_all = dotspool.tile([128, 512], F32, name="dots")
            dots_ps = [dots_all[:, 128 * p: 128 * p + 128] for p in range(4)]

            # block-diagonal P^T per head pair; off-diag blocks stay zero
            mhbd = [smx.tile([128, 128], F16, name=f"mhbd{p}") for p in range(4)]
            for p in range(4):
                nc.vector.memset(mhbd[p][:, :], 0.0)

            # PE warmup during the input-load head: back-to-back matmuls flip
            # the HAM clock gate to 8/8 (2.4 GHz) and keep it there until the
            # first projection matmuls are ready — any >3.4us PE-idle gap
            # re-throttles, and ~80%-busy phase A never re-warms it. Results
            # land in dots PSUM and are overwritten by the g==0 matmuls
            # (hw bits cleared by its start=True).
            for _ in range(34):
                nc.tensor.matmul(
                    dots_all[:, :],
                    lhsT=warmsrc[:, 0:128],
                    rhs=warmsrc[:, :],
                    start=True,
                    stop=True,
                )

            # ---------------- Phase A ----------------
            vdp_v = vdp.rearrange("p (r n) -> p r n", r=4)
            with (
                tc.tile_pool(name="xq", bufs=2) as xpool,
                tc.tile_pool(name="slots", bufs=5) as slotpool,
                tc.tile_pool(name="projp", bufs=4, space="PSUM") as projpool,
                tc.tile_pool(name="vtrp", bufs=3, space="PSUM") as vtrpool,
            ):
                slots = {}

                def ecopy(k, dst, src):
                    # PSUM evacuation: only DVE/ACT can read PSUM
                    if k % 2 == 0:
                        nc.vector.tensor_copy(dst, src)
                    else:
                        nc.scalar.copy(dst, src)

                def consume(g):
                    # dots + v-transpose for a group whose slot is fully evac'd
                    slot = slots.pop(g)
                    vt = vtrpool.tile([128, 512], F16, name="vt")
                    for pr in range(4):
                        qs = slot[:, 128 * pr: 128 * pr + 128]
                        ks = slot[:, 512 + 128 * pr: 512 + 128 * pr + 128]
                        vs = slot[:, 1024 + 128 * pr: 1024 + 128 * pr + 128]
                        nc.tensor.matmul(
                            dots_ps[pr][:, :],
                            lhsT=qs,
                            rhs=ks,
                            start=(g == 0 and pr == 0),
                            stop=(g == n_groups - 1),
                        )
                        nc.tensor.transpose(
                            vt[:, pr * 128:(pr + 1) * 128], vs, ident_sb[:, :]
                        )
                    voff = g * 128
                    ecopy(g + 1, vdp_v[:, :, voff:voff + 128], vt[:, :])

                sq = 8 * ql // 4  # sub-load columns (4 per quarter)
                for q in range(NQ):
                    xq = xpool.tile([65, 8 * ql], F16, name="xq")
                    # t-major DRAM layout: 4 sub-loads per quarter so the
                    # first tile groups can start before the quarter lands
                    for tq in range(4):
                        nc.sync.dma_start(
                            out=xq[:, tq * sq:(tq + 1) * sq],
                            in_=x_ap[:, q * 8 * ql + tq * sq:
                                     q * 8 * ql + (tq + 1) * sq],
                        )
                    for t0 in range(tpq):
                        g = q * tpq + t0
                        tq, t0r = t0 // 4, t0 % 4
                        # slot cols: r*512 + head*64 + i*8 + alpha (alpha contiguous)
                        slot = slotpool.tile([128, 1536], F16, name="slot")
                        slot_sc = slot.rearrange(
                            "p (r h i a) -> p i r h a", r=3, h=8, i=8, a=8
                        )
                        slots[g] = slot
                        for ip in range(4):  # chunk pairs (2i, 2i+1)
                            pp = projpool.tile([128, 384], F32, name="pp")
                            for c in range(2):
                                i = 2 * ip + c
                                x0 = tq * sq + i * 512 + t0r * 128
                                nc.tensor.matmul(
                                    pp[:, c * 192:(c + 1) * 192],
                                    lhsT=xq[:, x0:x0 + 128],
                                    rhs=wqkv_sb[:, :],
                                    start=True,
                                    stop=True,
                                )
                            dst = slot_sc[:, 2 * ip: 2 * ip + 2, :, :, :]
                            ecopy(ip, dst, pp[:, :])
                        if g >= 3:
                            consume(g - 3)
                for g in (n_groups - 3, n_groups - 2, n_groups - 1):
                    consume(g)

            # ---------------- Softmax + output ----------------
            with (
                tc.tile_pool(name="mhp", bufs=1, space="PSUM") as mhpool,
                tc.tile_pool(name="finp", bufs=5, space="PSUM") as finpool,
                tc.tile_pool(name="outs", bufs=4) as outpool,
            ):
                # keep the PE hot across the softmax gap so the finals run
                # at 2.4 GHz (issued right after the last dots matmul)
                warm = mhpool.tile([128, 256], F32, name="warm")
                for _ in range(26):
                    nc.tensor.matmul(
                        warm[:, :],
                        lhsT=warmsrc[:, 0:128],
                        rhs=warmsrc[:, 0:256],
                        start=True,
                        stop=True,
                    )

                negmax = smx.tile([128, 4], F32)
                rowsum = smx.tile([128, 4], F32)
                exps = smx.tile([128, 4 * 64], F16)
                wots = smx.tile([128, 4 * 64], F16)
                mh_ps = mhpool.tile([128, 4 * 64], F32)

                def hpb(h):
                    return h // 2, (h % 2) * 64

                # batched per-op issue so the cross-engine chain pipelines
                for h in range(HEADS):
                    pr, b = hpb(h)
                    nc.vector.reduce_max(
                        negmax[b:b + 64, pr:pr + 1],
                        dots_ps[pr][b:b + 64, b:b + 64],
                        axis=mybir.AxisListType.X, negate=True,
                    )
                for h in range(HEADS):
                    pr, b = hpb(h)
                    nc.scalar.activation(
                        exps[b:b + 64, pr * 64:(pr + 1) * 64],
                        dots_ps[pr][b:b + 64, b:b + 64],
                        mybir.ActivationFunctionType.Exp,
                        bias=negmax[b:b + 64, pr:pr + 1],
                        scale=1.0,
                        accum_out=rowsum[b:b + 64, pr:pr + 1],
                    )
                recip = smx.tile([128, 4], F32)
                for h in range(HEADS):
                    pr, b = hpb(h)
                    nc.vector.reciprocal(
                        recip[b:b + 64, pr:pr + 1], rowsum[b:b + 64, pr:pr + 1]
                    )
                for h in range(HEADS):
                    pr, b = hpb(h)
                    # 1/rowsum folded into the per-head copy of Wo^T
                    nc.vector.tensor_scalar_mul(
                        wots[b:b + 64, pr * 64:(pr + 1) * 64],
                        wot_sb[b:b + 64, :],
                        recip[b:b + 64, pr:pr + 1],
                    )
                for pr in range(4):
                    for s in range(2):
                        b = s * 64
                        nc.tensor.matmul(
                            mh_ps[b:b + 64, pr * 64:(pr + 1) * 64],
                            lhsT=exps[b:b + 64, pr * 64:(pr + 1) * 64],
                            rhs=wots[b:b + 64, pr * 64:(pr + 1) * 64],
                            start=True,
                            stop=True,
                        )
                        src = mh_ps[b:b + 64, pr * 64:(pr + 1) * 64]
                        dst = mhbd[pr][b:b + 64, b:b + 64]
                        if s == 0:
                            nc.vector.tensor_copy(dst, src)
                        else:
                            nc.scalar.copy(dst, src)

                    outsb = outpool.tile([128, bl], F16, name="outsb")
                    for s5 in range(bl // 512):
                        fp_ = finpool.tile([128, 512], F32, name="fp_")
                        n0 = pr * bl + s5 * 512
                        nc.tensor.matmul(
                            fp_[:, :],
                            lhsT=mhbd[pr][:, :],
                            rhs=vdp[:, n0:n0 + 512],
                            start=True,
                            stop=True,
                        )
                        dst = outsb[:, s5 * 512:(s5 + 1) * 512]
                        if s5 % 2 == 0:
                            nc.vector.tensor_copy(dst, fp_[:, :])
                        else:
                            nc.scalar.copy(dst, fp_[:, :])
                    # 4 SWDGE stores of 32 descriptors each: the packet
                    # rotation spreads consecutive calls across SDMA engines
                    for j in range(4):
                        s, o0 = j // 2, (j % 2) * 32
                        nc.gpsimd.dma_start(
                            out=out_v[pr, s, o0:o0 + 32, :],
                            in_=outsb[64 * s + o0: 64 * s + o0 + 32, :],
                        )

    nc.compile()
    return nc


_NC_CACHE = {}


def _get_nc(hw=HW):
    if hw not in _NC_CACHE:
        _NC_CACHE[hw] = _build_kernel(hw)
    return _NC_CACHE[hw]


def _host_inputs(Wq, bq, Wk, bk, Wv, bv, Wo):
    scale = 64 ** -0.5
    wqkv = np.zeros((65, 192), np.float16)
    wqkv[:64, 0:64] = (Wq.T * scale).astype(np.float16)
    wqkv[64, 0:64] = (bq * scale).astype(np.float16)
    wqkv[:64, 64:128] = Wk.T.astype(np.float16)
    wqkv[64, 64:128] = bk.astype(np.float16)
    wqkv[:64, 128:192] = Wv.T.astype(np.float16)
    wqkv[64, 128:192] = bv.astype(np.float16)
    # kernel uses c' = i*8 + alpha ordering; original c = alpha*8 + i
    pi = np.array([(c % 8) * 8 + c // 8 for c in range(64)])
    wotp = Wo.T[pi]
    wot = np.concatenate([wotp, wotp], axis=0).astype(np.float32)
    ident = np.eye(128, dtype=np.float16)
    return wqkv, wot, ident


def kernel(x, Wq, bq, Wk, bk, Wv, bv, Wo):
    global LAST_RESULTS
    B = x.shape[0]
    hw = x.shape[2] * x.shape[3]
    nc = _get_nc(hw)
    wqkv, wot, ident = _host_inputs(Wq, bq, Wk, bk, Wv, bv, Wo)

    ql = hw // HEADS // NQ
    in_maps = []
    for bidx in range(B):
        x65 = np.empty((65, hw), np.float16)
        x65[:64] = x[bidx].reshape(64, hw)
        x65[64] = 1.0
        # (q, tq, i, 512) column order: each quarter is 4 contiguous
        # sub-loads of 8KB-per-partition descriptors
        x65 = np.ascontiguousarray(
            x65.reshape(65, HEADS, NQ, 4, ql // 4).transpose(0, 2, 3, 1, 4)
        ).reshape(65, hw)
        in_maps.append({"x": x65, "wqkv": wqkv, "wot": wot, "ident": ident})

    trace = bool(os.environ.get("KERNEL_TRACE"))
    res = run_bass_kernel_spmd(
        nc, in_maps, core_ids=list(range(B)), trace=trace
    )
    LAST_RESULTS = res
    out = np.stack(
        [res.results[bidx]["out"].reshape(64, HEADS, hw // HEADS)
         for bidx in range(B)]
    ).astype(np.float32)
    return out
